# revision 26
# baseline (speedup 1.0000x reference)
"""Depthwise causal Conv1D (B=4, C=4096, L=4096, K=4) on 8 trn2 NeuronCores.

Sharding: channel-parallel — core i owns channels [i*512, (i+1)*512);
depthwise conv has no cross-channel interaction, so no communication.

HBM-bandwidth bound, so I/O is fp16 (harness gate is 2e-2; fp16 keeps max
rel err ~1e-3): host converts x, device computes/stores fp16, host upcasts.
~32.8 MB/core HBM traffic instead of ~67 MB fp32.

Host-packed batch layout: L + PAD = LOUT = 4099, so all 4 batches pack into
one padded row per channel: [3 zeros | b0 | 3 zeros | b1 | ...] (width
3 + 4*4099 = 16399). The shared zero gaps double as trailing/leading pad,
and out[m] = sum_t w_t * XP[m+t] holds globally over m in [0, 4*4099) —
one 4-tap FIR across the packed row, no per-batch edges on device.

Per-core: channels on partitions (4 groups of 128), packed time on the
free dim. Engine split per group (cols [0, N), N=16396):

  V region, cols [0, v_w): DVE ONLY — tap0 as tensor_scalar with the
    bias in the second scalar slot (TS runs ~0.4 ns/col in packed mode,
    alignment-insensitive — measured), taps 1-3 as TS products + a TT
    add tree. Fully decoupled from ScalarE/PE so no cross-engine
    program-order serialization.
  P region, cols [v_w, N): PE all 4 taps as diag-weight fp16 matmuls,
    tap-outer over 4x512-col chunks into a 4-bank [128,2048] fp32 PSUM
    tile (double-buffered = all 8 banks); ScalarE merges 2048 cols per
    activation (bias fused).

HAM clock gate: PE defaults to 1.2 GHz and reaches 2.4 GHz only after
~3.4us of sustained busy; any >3.4us PE idle gap re-throttles. P-region
input loads are issued first per group and prefetch runs 2 groups deep
so PE never idles at group boundaries.

DMA packet hygiene: per-partition row segments sized to 8KB multiples
where possible (fp16: 4096-col chunks) so HWDGE descriptors are full
packets. Loads + V stores on the SP (sync) HWDGE, P stores on ScalarE's.
"""

import numpy as np

import concourse.bass as bass
import concourse.tile as tile
from concourse import bacc, mybir
from concourse.bass_utils import run_bass_kernel_spmd

B, C, L, K = 4, 4096, 4096, 4
PAD = K - 1
LOUT = L + PAD  # 4099
NCORES = 8
CS = C // NCORES  # 512 channels per core
N = B * LOUT  # 16396 packed output cols
W = PAD + N  # 16399 packed input cols
DT = mybir.dt.float32
DT16 = mybir.dt.float16

_AF = mybir.ActivationFunctionType
_OP = mybir.AluOpType


def build_nc(
    cs=CS,
    n=N,
    k=K,
    pe_cols=10752,
    pe_chunk=512,
    pe_sweep=4,  # chunks per full PSUM sweep = 2048 cols = 4 banks
    n_v_chunks=1,
    n_v_chunks_last=2,
    p_store_cols=4096,
    x_bufs=3,
    o_bufs=2,
):
    """Per-core Bass program over the host-packed fp16 layout.

    x_d  [cs, W]    fp16  packed zero-stuffed input rows
    wb_d [128, 5*ng] fp32 per-(partition,group) consts [w0..w3, bias]
    o_d  [cs, N]    fp16  packed output rows
    """
    pad = k - 1
    w_cols = pad + n
    ng = cs // 128
    sweep_cols = pe_chunk * pe_sweep
    assert pe_cols % pe_chunk == 0
    # Full sweeps of pe_sweep chunks plus one trailing partial sweep.
    n_chunks_p = pe_cols // pe_chunk
    sweeps = []  # (col offset within P region, n chunks)
    off = 0
    while off < n_chunks_p:
        m = min(pe_sweep, n_chunks_p - off)
        sweeps.append((off * pe_chunk, m))
        off += m
    n_sweeps = len(sweeps)
    # V region first (cols [0, v_w)), P region after (cols [v_w, n)).
    v_w = n - pe_cols
    p_lo = v_w

    nc = bacc.Bacc("TRN2", target_bir_lowering=False, debug=False, num_devices=NCORES)
    x_d = nc.dram_tensor("x", [cs, w_cols], DT16, kind="ExternalInput").ap()
    wb_d = nc.dram_tensor("wb", [128, (k + 1) * ng], DT, kind="ExternalInput").ap()
    eye_d = nc.dram_tensor("eye", [128, 128], DT16, kind="ExternalInput").ap()
    o_d = nc.dram_tensor("out", [cs, n], DT16, kind="ExternalOutput").ap()

    with tile.TileContext(nc) as tc:
        with (
            tc.tile_pool(name="consts", bufs=1) as cpool,
            tc.tile_pool(name="xs", bufs=x_bufs) as xpool,
            # Separate V/P output tiles: DVE only ever writes vo (3 bufs
            # decouple its tap0 TS from V-store completion two groups
            # back), ScalarE merges write po.
            tc.tile_pool(name="vos", bufs=3) as vopool,
            tc.tile_pool(name="pos", bufs=o_bufs) as popool,
            tc.tile_pool(name="tmps", bufs=1) as tpool,
            tc.tile_pool(name="ps", bufs=2, space="PSUM") as ppool,
        ):
            consts = [None] * ng
            diags = {}

            def emit_consts():
                wbt = cpool.tile([128, (k + 1) * ng], DT, tag="wb")
                nc.sync.dma_start(wbt[:], wb_d[:])
                ident = cpool.tile([128, 128], DT16, tag="eye")
                nc.sync.dma_start(ident[:], eye_d[:])
                for g in range(ng):
                    consts[g] = wbt[:, (k + 1) * g : (k + 1) * (g + 1)]
                    for t in range(k):
                        dg = cpool.tile([128, 128], DT16, tag=f"d{g}_{t}")
                        nc.vector.tensor_scalar(
                            out=dg[:], in0=ident[:],
                            scalar1=consts[g][:, t : t + 1],
                            scalar2=None, op0=_OP.mult,
                        )
                        diags[(g, t)] = dg

            # Load chunks: a small first P chunk (PE's first sweep starts
            # ~3us earlier, HAM warms sooner), the next P sweep's worth,
            # then the whole V region (DVE ramps), then the rest of P.
            # Per-row segments stay >=4KB / 8KB-multiple where possible.
            l_edges = [
                (p_lo, p_lo + 2052),
                (p_lo + 2052, p_lo + 6148),
                (0, v_w),
                (p_lo + 6148, w_cols),
            ]

            xts = [None] * ng

            def load_group(g):
                c0 = g * 128
                xt = xpool.tile([128, w_cols], DT16, tag="x")
                for a, b in l_edges:
                    nc.sync.dma_start(xt[:, a:b], x_d[c0 : c0 + 128, a:b])
                xts[g] = xt

            emit_consts()  # tiny DMAs; diags build before first MM
            load_group(0)
            if ng > 1:
                load_group(1)
            for g in range(ng):
                c0 = g * 128
                last = g == ng - 1
                # Software-pipelined loads, two groups ahead (x_bufs=3):
                # emitted BEFORE group g's V store so the Sync queue never
                # blocks loads on DVE's last TT (measured 3-8us DVE gaps
                # per group boundary otherwise).
                if g + 2 < ng:
                    load_group(g + 2)
                xt = xts[g]
                nvc = n_v_chunks_last if last else n_v_chunks
                vcw = (v_w // nvc + 2) & ~1  # even chunk width
                vo = vopool.tile([128, v_w], DT16, tag="vo")
                po = popool.tile([128, pe_cols], DT16, tag="po")
                ct = consts[g]

                def v_chunk(ci):
                    j0 = ci * vcw
                    j1 = min(j0 + vcw, v_w)
                    tmax = (v_w + 2) & ~1
                    t1 = tpool.tile([128, tmax], DT16, tag="t1")
                    t2 = tpool.tile([128, tmax], DT16, tag="t2")
                    m = j1 - j0
                    # tap0 + bias straight into ot (TS 2-scalar slot).
                    # Keep this on DVE: routing it through ScalarE chains
                    # DVE behind ScalarE's merge-paced FIFO and measured a
                    # 20us pipeline stall (v4 post-mortem).
                    nc.vector.tensor_scalar(
                        out=vo[:, j0:j1], in0=xt[:, j0:j1],
                        scalar1=ct[:, 0:1], scalar2=ct[:, k : k + 1],
                        op0=_OP.mult, op1=_OP.add,
                    )
                    nc.vector.tensor_scalar(
                        out=t1[:, :m], in0=xt[:, j0 + 1 : j1 + 1],
                        scalar1=ct[:, 1:2], scalar2=None, op0=_OP.mult,
                    )
                    nc.vector.tensor_scalar(
                        out=t2[:, :m], in0=xt[:, j0 + 2 : j1 + 2],
                        scalar1=ct[:, 2:3], scalar2=None, op0=_OP.mult,
                    )
                    nc.vector.tensor_tensor(
                        out=t1[:, :m], in0=t1[:, :m], in1=t2[:, :m], op=_OP.add
                    )
                    nc.vector.tensor_scalar(
                        out=t2[:, :m], in0=xt[:, j0 + 3 : j1 + 3],
                        scalar1=ct[:, 3:4], scalar2=None, op0=_OP.mult,
                    )
                    nc.vector.tensor_tensor(
                        out=t1[:, :m], in0=t1[:, :m], in1=t2[:, :m], op=_OP.add
                    )
                    nc.vector.tensor_tensor(
                        out=vo[:, j0:j1], in0=vo[:, j0:j1], in1=t1[:, :m], op=_OP.add
                    )

                def p_sweep(si):
                    soff, nch = sweeps[si]
                    s0 = p_lo + soff
                    scols = nch * pe_chunk
                    pts = ppool.tile([128, sweep_cols], DT, tag="p")
                    for t in range(k):
                        for c in range(nch):
                            m0 = s0 + c * pe_chunk
                            nc.tensor.matmul(
                                pts[:, c * pe_chunk : (c + 1) * pe_chunk],
                                lhsT=diags[(g, t)][:],
                                rhs=xt[:, m0 + t : m0 + t + pe_chunk],
                                start=(t == 0), stop=(t == k - 1),
                            )
                    nc.scalar.activation(
                        po[:, soff : soff + scols], pts[:, :scols], _AF.Identity,
                        bias=ct[:, k : k + 1], scale=1.0,
                    )
                    # P stores on ScalarE's HWDGE; 4096-col (8KB) chunks,
                    # remainder merged into the final store.
                    done = soff + scols
                    if done == pe_cols:
                        st0 = (pe_cols // p_store_cols - 1) * p_store_cols
                        nc.scalar.dma_start(
                            o_d[c0 : c0 + 128, p_lo + st0 : n], po[:, st0:]
                        )
                    elif done % p_store_cols == 0 and done < (
                        pe_cols // p_store_cols
                    ) * p_store_cols:
                        st0 = done - p_store_cols
                        nc.scalar.dma_start(
                            o_d[c0 : c0 + 128, p_lo + st0 : p_lo + done],
                            po[:, st0:done],
                        )

                # Interleave emission: V chunks between early P sweeps so
                # DVE's loads gate nothing and PE is never starved. The
                # last group runs V first so its store drains under the
                # remaining PE work instead of extending the tail.
                # V stores ride the Sync queue (loads of g+1 were already
                # emitted above, so the store blocks nothing). The last
                # group runs V first and stores per chunk so the drain
                # overlaps the remaining PE work.
                if last:
                    for i in range(nvc):
                        v_chunk(i)
                        j0, j1 = i * vcw, min((i + 1) * vcw, v_w)
                        nc.sync.dma_start(o_d[c0 : c0 + 128, j0:j1], vo[:, j0:j1])
                    for i in range(n_sweeps):
                        p_sweep(i)
                else:
                    for i in range(max(nvc, n_sweeps)):
                        if i < nvc:
                            v_chunk(i)
                        if i < n_sweeps:
                            p_sweep(i)
                    nc.sync.dma_start(o_d[c0 : c0 + 128, 0:v_w], vo[:, 0:v_w])
    nc.compile()
    return nc


_cached = {}


def _get_nc(**kw):
    key = tuple(sorted(kw.items()))
    if key not in _cached:
        _cached[key] = build_nc(**kw)
    return _cached[key]


def _pack_inputs(x, kernel, bias):
    """Host-side: fp16 packed x rows + per-core input maps."""
    w = np.asarray(kernel, dtype=np.float32).reshape(K, C)
    bvec = np.asarray(bias, dtype=np.float32).reshape(C)
    wb = np.concatenate([w.T, bvec[:, None]], axis=1).astype(np.float32)  # [C,5]

    x16 = np.asarray(x).astype(np.float16)  # [B, C, L]
    xp = np.zeros((C, W), dtype=np.float16)
    for bi in range(B):
        xp[:, bi * LOUT + PAD : bi * LOUT + PAD + L] = x16[bi]

    eye = np.eye(128, dtype=np.float16)
    ng = CS // 128
    in_maps = []
    for i in range(NCORES):
        sl = slice(i * CS, (i + 1) * CS)
        # [128, 5*ng]: group g of this core occupies cols [5g, 5g+5)
        wbc = np.ascontiguousarray(
            wb[sl].reshape(ng, 128, K + 1).transpose(1, 0, 2).reshape(128, -1)
        )
        in_maps.append(
            {
                "x": np.ascontiguousarray(xp[sl, :]),
                "wb": wbc,
                "eye": eye,
            }
        )
    return in_maps


def run(x, kernel, bias, trace=False, build_kw=None, **kwargs):
    """Shard, run on 8 cores, gather. Returns (out, BassKernelResults)."""
    in_maps = _pack_inputs(x, kernel, bias)
    nc = _get_nc(**(build_kw or {}))
    bkr = run_bass_kernel_spmd(
        nc, in_maps, core_ids=list(range(NCORES)), trace=trace, **kwargs
    )
    outs = [
        r["out"].reshape(CS, B, LOUT).transpose(1, 0, 2).astype(np.float32)
        for r in bkr.results
    ]
    return np.concatenate(outs, axis=1), bkr


def kernel(x, kernel, bias):
    import os

    prev = os.environ.get("BASS_NEVER_TRACE")
    os.environ["BASS_NEVER_TRACE"] = "1"  # keep the runner off the NTFF path
    try:
        out, _ = run(x, kernel, bias)
    finally:
        if prev is None:
            os.environ.pop("BASS_NEVER_TRACE", None)
        else:
            os.environ["BASS_NEVER_TRACE"] = prev
    return out
